# revision 3
# baseline (speedup 1.0000x reference)
"""Trainium2 Bass kernel for nn_Algebraic_interval: t-norm feature expansion.

For each input x in {xl, xu} of shape [65536, 16], computes
  out = concat([x, prod(x[:, idx2], -1), prod(x[:, idx3], -1)], axis=1)
over all C(16,2)=120 pair and C(16,3)=560 triple column combinations,
giving two [65536, 696] outputs.

Strategy (pure data parallel over 8 cores, 8192 rows each):
  - Products are computed as exp(G^T @ ln(x)): TensorE gathers/sums the
    logs through a static 0/1 combination matrix G, ScalarE does ln/exp.
  - fp32 matmuls run at 1/4 rate on the PE (two-pass decomposition), so
    ln(x) is split into three bf16 components h1+h2+h3 (~24 mantissa
    bits) stacked along the contraction dim: K=96 = 3 passes x 32
    features (16 xl + 16 xu interleaved; zero rows in G select the
    tensor). One full-rate bf16 matmul per 512-column chunk then
    reconstructs G @ ln(x) in fp32 PSUM exactly (G entries are exact in
    bf16, so products are exact; only the 3-way bf16 split truncates).
  - Inputs are clamped to >=1e-38 so ln stays finite; a true 0 input
    gives exp(sum <= -87.4) -> underflow to 0 = the exact product.
  - One [128, 2x696] exp per 128-row tile writes both outputs' rows into
    an SBUF slab; slabs of 4 tiles stream out via 1.4 MB DMAs.

Host-side: inputs are pre-transposed to feature-major xt[32, 8192]
(partition p<16: xl feature p; p>=16: xu feature p-16) per core.
"""

import itertools
import numpy as np

N_COLS = 16
B_FULL = 65536
N_CORES = 8
B_CORE = B_FULL // N_CORES          # 8192
PAIRS = list(itertools.combinations(range(N_COLS), 2))    # 120
TRIPLES = list(itertools.combinations(range(N_COLS), 3))  # 560
N_OUT = N_COLS + len(PAIRS) + len(TRIPLES)                # 696
TILES_PER_CORE = B_CORE // 128      # 64
TILES_PER_SLAB = 4
N_SLABS = TILES_PER_CORE // TILES_PER_SLAB  # 16
# matmul output chunking over the 2*696 concatenated columns (PSUM banks)
CHUNKS = [(0, 512), (512, 512), (1024, 368)]

_CACHED = {}


def _make_g() -> np.ndarray:
    """[96, 2*696] bf16 0/1 matrix, 3 vertical copies of [32, 1392].

    Rows (within a 32-block): 0..15 select xl features, 16..31 xu.
    Columns 0:696 are out_l (16 singles | 120 pairs | 560 triples, lex
    order), columns 696:1392 are out_u.
    """
    import ml_dtypes

    g = np.zeros((32, 2 * N_OUT), dtype=np.float32)
    for half, row0 in ((0, 0), (1, 16)):
        c0 = half * N_OUT
        for j in range(N_COLS):
            g[row0 + j, c0 + j] = 1.0
        for idx, pair in enumerate(PAIRS):
            for f in pair:
                g[row0 + f, c0 + N_COLS + idx] = 1.0
        for idx, tri in enumerate(TRIPLES):
            for f in tri:
                g[row0 + f, c0 + N_COLS + len(PAIRS) + idx] = 1.0
    return np.tile(g, (3, 1)).astype(ml_dtypes.bfloat16)


def _build_program():
    import concourse.bacc as bacc
    import concourse.mybir as mybir
    import concourse.tile as tile
    from concourse.bass import MemorySpace

    f32 = mybir.dt.float32
    bf16 = mybir.dt.bfloat16
    Act = mybir.ActivationFunctionType
    nc = bacc.Bacc("TRN2", target_bir_lowering=False, debug=False)

    xt = nc.dram_tensor("xt", [32, B_CORE], f32, kind="ExternalInput")
    out_l = nc.dram_tensor("out_l", [B_CORE, N_OUT], f32, kind="ExternalOutput")
    out_u = nc.dram_tensor("out_u", [B_CORE, N_OUT], f32, kind="ExternalOutput")
    gm = nc.inline_tensor(_make_g(), name="gmat")

    with tile.TileContext(nc) as tc:
        with (
            tc.tile_pool(name="const", bufs=1) as const_pool,
            tc.tile_pool(name="inp", bufs=1) as inp_pool,
            tc.tile_pool(name="slab", bufs=3) as slab_pool,
            tc.tile_pool(name="psum", bufs=2, space=MemorySpace.PSUM) as psum_pool,
        ):
            gm_sb = const_pool.tile([96, 2 * N_OUT], bf16)
            nc.sync.dma_start(gm_sb[:], gm[:])

            xt_sb = inp_pool.tile([32, B_CORE], f32)
            nc.sync.dma_start(xt_sb[:], xt[:])

            # ln(max(x, 1e-38)), then 3-way bf16 split of the logs:
            # h1=bf16(ln), h2=bf16(ln-h1), h3=bf16(ln-h1-h2).
            # DVE ops cannot cross partitions, so h2/h3 are computed on
            # partitions 0:32 and DMA'd into the stacked weight buffer.
            r1 = inp_pool.tile([32, B_CORE], f32)
            nc.vector.tensor_scalar_max(r1[:], xt_sb[:], 1e-38)
            nc.scalar.activation(xt_sb[:], r1[:], Act.Ln)  # xt_sb := ln
            w = inp_pool.tile([96, B_CORE], bf16)
            nc.vector.tensor_copy(w[0:32, :], xt_sb[:])
            nc.vector.tensor_sub(r1[:], xt_sb[:], w[0:32, :])
            h2 = inp_pool.tile([32, B_CORE], bf16)
            nc.vector.tensor_copy(h2[:], r1[:])
            h3 = inp_pool.tile([32, B_CORE], bf16)
            nc.vector.tensor_sub(h3[:], r1[:], h2[:])
            nc.sync.dma_start(w[32:64, :], h2[:])
            nc.sync.dma_start(w[64:96, :], h3[:])

            ov_l = out_l.ap().rearrange(
                "(s q p) c -> s p q c", p=128, q=TILES_PER_SLAB
            )
            ov_u = out_u.ap().rearrange(
                "(s q p) c -> s p q c", p=128, q=TILES_PER_SLAB
            )

            for s in range(N_SLABS):
                slab = slab_pool.tile([128, 2, TILES_PER_SLAB, N_OUT], f32)
                for q in range(TILES_PER_SLAB):
                    t = s * TILES_PER_SLAB + q
                    o = t * 128
                    lhsT = w[:, o : o + 128]
                    S = psum_pool.tile([128, 1536], f32)
                    for c0, cw in CHUNKS:
                        nc.tensor.matmul(
                            S[:, c0 : c0 + cw],
                            lhsT,
                            gm_sb[:, c0 : c0 + cw],
                        )
                    nc.scalar.activation(
                        slab[:, :, q, :],
                        S[:, 0 : 2 * N_OUT].rearrange(
                            "p (two c) -> p two c", two=2
                        ),
                        Act.Exp,
                    )
                nc.sync.dma_start(ov_l[s], slab[:, 0])
                nc.sync.dma_start(ov_u[s], slab[:, 1])

    nc.compile()
    return nc


def kernel(xl, xu):
    from concourse.bass_utils import run_bass_kernel_spmd

    xl = np.asarray(xl, dtype=np.float32)
    xu = np.asarray(xu, dtype=np.float32)

    if "nc" not in _CACHED:
        _CACHED["nc"] = _build_program()
    nc = _CACHED["nc"]

    in_maps = []
    for i in range(N_CORES):
        lo, hi = i * B_CORE, (i + 1) * B_CORE
        xt = np.concatenate([xl[lo:hi].T, xu[lo:hi].T], axis=0)
        in_maps.append({"xt": np.ascontiguousarray(xt)})

    res = run_bass_kernel_spmd(nc, in_maps, list(range(N_CORES)))

    full_l = np.concatenate([res.results[i]["out_l"] for i in range(N_CORES)], axis=0)
    full_u = np.concatenate([res.results[i]["out_u"] for i in range(N_CORES)], axis=0)
    return full_l, full_u


# revision 6
# speedup vs baseline: 1.1224x; 1.1224x over previous
"""Trainium2 Bass kernel for nn_Algebraic_interval: t-norm feature expansion.

For each input x in {xl, xu} of shape [65536, 16], computes
  out = concat([x, prod(x[:, idx2], -1), prod(x[:, idx3], -1)], axis=1)
over all C(16,2)=120 pair and C(16,3)=560 triple column combinations,
giving two [65536, 696] outputs.

Strategy (pure data parallel over 8 cores, 8192 rows each):
  - Products are computed as exp(G^T @ ln(x)): TensorE gathers/sums the
    logs through a static 0/1 combination matrix G, ScalarE does ln/exp.
  - fp32 matmuls run at 1/4 rate on the PE (two-pass decomposition), so
    ln(x) is split into three bf16 components h1+h2+h3 (~24 mantissa
    bits) stacked along the contraction dim: K=96 = 3 passes x 32
    features (16 xl + 16 xu interleaved; zero rows in G select the
    tensor). One full-rate bf16 matmul per 512-column chunk then
    reconstructs G @ ln(x) in fp32 PSUM exactly (G entries are exact in
    bf16, so products are exact; only the 3-way bf16 split truncates).
  - Inputs are clamped to >=1e-38 so ln stays finite; a true 0 input
    gives exp(sum <= -87.4) -> underflow to 0 = the exact product.
  - One [128, 2x696] exp per 128-row tile writes both outputs' rows into
    an SBUF slab; slabs of 4 tiles stream out via 1.4 MB DMAs.

Host-side: inputs are pre-transposed to feature-major xt[32, 8192]
(partition p<16: xl feature p; p>=16: xu feature p-16) per core.
"""

import itertools
import numpy as np

N_COLS = 16
B_FULL = 65536
N_CORES = 8
B_CORE = B_FULL // N_CORES          # 8192
PAIRS = list(itertools.combinations(range(N_COLS), 2))    # 120
TRIPLES = list(itertools.combinations(range(N_COLS), 3))  # 560
N_OUT = N_COLS + len(PAIRS) + len(TRIPLES)                # 696
TILES_PER_CORE = B_CORE // 128      # 64
TILES_PER_SLAB = 4
N_SLABS = TILES_PER_CORE // TILES_PER_SLAB  # 16
# matmul output chunking over the 2*696 concatenated columns (PSUM banks)
CHUNKS = [(0, 512), (512, 512), (1024, 368)]

_CACHED = {}


def _make_g() -> np.ndarray:
    """[96, 2*696] bf16 0/1 matrix, 3 vertical copies of [32, 1392].

    Rows (within a 32-block): 0..15 select xl features, 16..31 xu.
    Columns 0:696 are out_l (16 singles | 120 pairs | 560 triples, lex
    order), columns 696:1392 are out_u.
    """
    import ml_dtypes

    g = np.zeros((32, 2 * N_OUT), dtype=np.float32)
    for half, row0 in ((0, 0), (1, 16)):
        c0 = half * N_OUT
        for j in range(N_COLS):
            g[row0 + j, c0 + j] = 1.0
        for idx, pair in enumerate(PAIRS):
            for f in pair:
                g[row0 + f, c0 + N_COLS + idx] = 1.0
        for idx, tri in enumerate(TRIPLES):
            for f in tri:
                g[row0 + f, c0 + N_COLS + len(PAIRS) + idx] = 1.0
    return np.tile(g, (3, 1)).astype(ml_dtypes.bfloat16)


def _build_program():
    import concourse.bacc as bacc
    import concourse.mybir as mybir
    import concourse.tile as tile
    from concourse.bass import MemorySpace

    f32 = mybir.dt.float32
    bf16 = mybir.dt.bfloat16
    Act = mybir.ActivationFunctionType
    nc = bacc.Bacc("TRN2", target_bir_lowering=False, debug=False)

    xt = nc.dram_tensor("xt", [32, B_CORE], f32, kind="ExternalInput")
    out_l = nc.dram_tensor("out_l", [B_CORE, N_OUT], f32, kind="ExternalOutput")
    out_u = nc.dram_tensor("out_u", [B_CORE, N_OUT], f32, kind="ExternalOutput")
    gm = nc.inline_tensor(_make_g(), name="gmat")

    with tile.TileContext(nc) as tc:
        with (
            tc.tile_pool(name="const", bufs=1) as const_pool,
            tc.tile_pool(name="inp", bufs=1) as inp_pool,
            tc.tile_pool(name="scratch", bufs=2) as scratch_pool,
            tc.tile_pool(name="slab", bufs=3) as slab_pool,
            tc.tile_pool(name="psum", bufs=2, space=MemorySpace.PSUM) as psum_pool,
        ):
            gm_sb = const_pool.tile([96, 2 * N_OUT], bf16)
            nc.sync.dma_start(gm_sb[:], gm[:])

            # ln(max(x, 1e-38)), then 3-way bf16 split of the logs:
            # h1=bf16(ln), h2=bf16(ln-h1), h3=bf16(ln-h1-h2).
            # DVE ops cannot cross partitions, so h2/h3 are computed on
            # partitions 0:32 and DMA'd into the stacked weight buffer.
            # Chunked along the batch dim so matmuls can start early.
            N_CHUNKS = 4
            CW = B_CORE // N_CHUNKS
            w_chunks = []
            for j in range(N_CHUNKS):
                cols = slice(j * CW, (j + 1) * CW)
                xt_sb = scratch_pool.tile([32, CW], f32, tag="xt_sb")
                nc.sync.dma_start(xt_sb[:], xt[:, cols])
                r1 = scratch_pool.tile([32, CW], f32, tag="r1")
                nc.vector.tensor_scalar_max(r1[:], xt_sb[:], 1e-38)
                nc.scalar.activation(xt_sb[:], r1[:], Act.Ln)  # := ln
                w = inp_pool.tile([96, CW], bf16, tag=f"w{j}")
                nc.vector.tensor_copy(w[0:32, :], xt_sb[:])
                nc.vector.tensor_sub(r1[:], xt_sb[:], w[0:32, :])
                h2 = scratch_pool.tile([32, CW], bf16, tag="h2")
                nc.vector.tensor_copy(h2[:], r1[:])
                h3 = scratch_pool.tile([32, CW], bf16, tag="h3")
                nc.vector.tensor_sub(h3[:], r1[:], h2[:])
                nc.sync.dma_start(w[32:64, :], h2[:])
                nc.sync.dma_start(w[64:96, :], h3[:])
                w_chunks.append(w)

            ov_l = out_l.ap().rearrange(
                "(s q p) c -> s p q c", p=128, q=TILES_PER_SLAB
            )
            ov_u = out_u.ap().rearrange(
                "(s q p) c -> s p q c", p=128, q=TILES_PER_SLAB
            )

            for s in range(N_SLABS):
                slab = slab_pool.tile([128, 2, TILES_PER_SLAB, N_OUT], f32)
                for q in range(TILES_PER_SLAB):
                    t = s * TILES_PER_SLAB + q
                    j = t // (TILES_PER_CORE // N_CHUNKS)
                    o = (t % (TILES_PER_CORE // N_CHUNKS)) * 128
                    lhsT = w_chunks[j][:, o : o + 128]
                    S = psum_pool.tile([128, 1536], f32)
                    for c0, cw in CHUNKS:
                        nc.tensor.matmul(
                            S[:, c0 : c0 + cw],
                            lhsT,
                            gm_sb[:, c0 : c0 + cw],
                        )
                    nc.scalar.activation(
                        slab[:, :, q, :],
                        S[:, 0 : 2 * N_OUT].rearrange(
                            "p (two c) -> p two c", two=2
                        ),
                        Act.Exp,
                    )
                nc.sync.dma_start(ov_l[s], slab[:, 0])
                nc.sync.dma_start(ov_u[s], slab[:, 1])

    nc.compile()
    return nc


def kernel(xl, xu):
    from concourse.bass_utils import run_bass_kernel_spmd

    xl = np.asarray(xl, dtype=np.float32)
    xu = np.asarray(xu, dtype=np.float32)

    if "nc" not in _CACHED:
        _CACHED["nc"] = _build_program()
    nc = _CACHED["nc"]

    in_maps = []
    for i in range(N_CORES):
        lo, hi = i * B_CORE, (i + 1) * B_CORE
        xt = np.concatenate([xl[lo:hi].T, xu[lo:hi].T], axis=0)
        in_maps.append({"xt": np.ascontiguousarray(xt)})

    res = run_bass_kernel_spmd(nc, in_maps, list(range(N_CORES)))

    full_l = np.concatenate([res.results[i]["out_l"] for i in range(N_CORES)], axis=0)
    full_u = np.concatenate([res.results[i]["out_u"] for i in range(N_CORES)], axis=0)
    return full_l, full_u
